# revision 6
# baseline (speedup 1.0000x reference)
"""Trainium2 Bass kernel for nn_ComputeAllAtomCoords.

Host strategy: residues are sorted by type (seq_index values with count > 0);
each active type is assigned a set of NeuronCores (data parallel over
residues). Each core processes a fixed capacity CAP of same-type residues, so
the per-type tables (RTs_in_base_frame / xyzs_in_base_frame / base_indices)
become shared matmul weights and the computation maps onto TensorE + DVE/ACT
with an fp16 datapath (fp32 PSUM accumulation). Any residues beyond the
device capacity (a handful when counts don't divide evenly) are computed
exactly on the host in numpy.

Device layout per core (CAP = 32 * NPG residues, NPG = 512):
  - residue slot r -> (g, n) = (r // NPG, r % NPG)
  - state planes T_fr (= RTF_fr^T): sbuf [128, 4*NPC] fp16 per chunk;
    partition (32*jb + g) holds matrix col j = JORD[jb]; free (4*n + i) =
    matrix row i.  JORD = [1,2,0,3] puts the rotX-active columns r1/r2 in
    partitions [0:64) (HW requires SBUF tensor_tensor inputs to share the
    base partition).
  - chain step s (torsion k = s+2): RTF' = RTF @ B @ rotX(alpha_k), i.e.
    T' = rotX^T (B^T T):
      m1    = matmul(lhsT_main, T)            PSUM fp32, all 128 rows
      psum2 = matmul(lhsT_swap, T)            PSUM fp32, rows = partner(r1,r2)
      T'    = fp16 copy of m1 (ScalarE)
      v = T'[0:64] * ct ; u = psum2 * stt ; T'[0:64] = v + u   (DVE)
    ct/stt are host-computed c,s factor tiles ([64, 8*NPG] fp16; stt rows
    [0:32) = +s for r1 rows, [32:64) = -s for r2 rows).
  - xyz: uniform batch schedule shared by all cores: slot b = (fr, ip, quad);
    out[32*a+g, n] = sum_j x[atom_a, j] * RTF_fr[ip, j] via one matmul per
    batch (lhsT block-diagonal over g), accumulated nowhere (one frame per
    slot), copied to fp16 and DMA'd out.
Outputs are SoA fp16 scratch; the host transposes/casts and scatters back.
"""
import sys
import numpy as np

for _p in ("/opt/trn_rl_repo",):
    if _p not in sys.path:
        sys.path.insert(0, _p)

import concourse.bass as bass
import concourse.bacc as bacc
import concourse.mybir as mybir
import concourse.tile as tile
from concourse.bass_utils import run_bass_kernel_spmd
from contextlib import ExitStack

F16 = mybir.dt.float16
F32 = mybir.dt.float32

G = 32
NPG = 512
CAP = G * NPG
NCHUNK = 2
NPC = NPG // NCHUNK
CHF = 4 * NPC
NSTEP = 8
JORD = [1, 2, 0, 3]
STEPS = [(0, 1), (1, 2), (2, 3), (3, 4), (3, 5), (5, 6), (6, 7), (6, 8)]
NFR = 9
XG = 4
NCORE = 8
EPS = 1e-6


def make_schedule(bi, active_types):
    per_type_atoms = {
        t: {fr: [a for a in range(34) if bi[t][a] == fr] for fr in range(NFR)}
        for t in active_types
    }
    slots = []
    per_type_quads = {t: {} for t in active_types}
    for fr in range(NFR):
        nq = max((len(per_type_atoms[t][fr]) + 3) // 4 for t in active_types)
        for qi in range(nq):
            for t in active_types:
                per_type_quads[t][(fr, qi)] = \
                    per_type_atoms[t][fr][qi * 4:qi * 4 + 4]
            for ip in range(3):
                slots.append((fr, ip, qi))
    while len(slots) % XG:
        slots.append((0, 0, 10**6))  # dead slot
    return slots, per_type_quads


def build_type_tables(slots, quads_t, rts_t, xyz_t):
    lhsT = np.zeros((NSTEP, 128, 128), np.float16)
    lhsT_sw = np.zeros((NSTEP, 128, 64), np.float16)
    eye = np.eye(G, dtype=np.float32)
    for s in range(NSTEP):
        B = np.asarray(rts_t[s + 2], np.float32)[np.ix_(JORD, JORD)]
        full = np.kron(B, eye).astype(np.float16)
        lhsT[s] = full
        # swap-mm: out rows [0:32) = r2 result, [32:64) = r1 result
        lhsT_sw[s] = full[:, np.r_[32:64, 0:32]]
    NB = len(slots)
    wx = np.zeros((NB, 128, 128), np.float16)
    rowmap = []
    gidx = np.arange(G)
    for b, (fr, ip, qi) in enumerate(slots):
        quad = quads_t.get((fr, qi), [])
        rows = []
        for a, atom in enumerate(quad):
            xv = np.asarray(xyz_t[atom], np.float32)
            for jb in range(4):
                wx[b, 32 * jb + gidx, 32 * a + gidx] = xv[JORD[jb]]
            rows.append((a, atom, ip))
        rowmap.append(rows)
    lhsT = np.ascontiguousarray(lhsT.transpose(1, 0, 2).reshape(128, -1))
    lhsT_sw = np.ascontiguousarray(lhsT_sw.transpose(1, 0, 2).reshape(128, -1))
    wx = np.ascontiguousarray(wx.transpose(1, 0, 2).reshape(128, -1))
    return lhsT, lhsT_sw, wx, rowmap


def pack_core_inputs(res_idx, alphas, Rs, Ts):
    al = np.asarray(alphas[0], np.float32)[res_idx]          # (CAP, 10, 2)
    a0 = al[:, 2:10, 0].reshape(G, NPG, 8)
    a1 = al[:, 2:10, 1].reshape(G, NPG, 8)
    nrm = np.sqrt(a0 * a0 + a1 * a1) + EPS
    c = (a0 / nrm).transpose(2, 0, 1)                         # (s, g, n)
    s_ = (a1 / nrm).transpose(2, 0, 1)
    ct = np.zeros((64, NSTEP * NPG), np.float32)
    stt = np.zeros((64, NSTEP * NPG), np.float32)
    for s in range(NSTEP):
        blk = slice(s * NPG, (s + 1) * NPG)
        ct[0:32, blk] = c[s]
        ct[32:64, blk] = c[s]
        stt[0:32, blk] = s_[s]        # r1 rows get +s (times r2 data)
        stt[32:64, blk] = -s_[s]      # r2 rows get -s (times r1 data)
    M0 = np.zeros((CAP, 4, 4), np.float32)
    M0[:, :3, :3] = np.asarray(Rs, np.float32)[res_idx]
    M0[:, :3, 3] = np.asarray(Ts, np.float32)[res_idx]
    M0[:, 3, 3] = 1.0
    t0 = M0.reshape(G, NPG, 4, 4).transpose(3, 0, 1, 2)[JORD] \
        .reshape(128, 4 * NPG)
    return {
        "t0": np.ascontiguousarray(t0.astype(np.float16)),
        "ct": np.ascontiguousarray(ct.astype(np.float16)),
        "stt": np.ascontiguousarray(stt.astype(np.float16)),
    }


def unpack_outputs(frames_buf, xyz_buf, nslots, rowmap):
    fb = np.asarray(frames_buf, np.float32).reshape(NFR, 4, G, NPG, 4)
    frames = np.empty((G, NPG, NFR, 4, 4), np.float32)
    for jb in range(4):
        frames[:, :, :, :, JORD[jb]] = fb[:, jb].transpose(1, 2, 0, 3)
    frames = frames.reshape(CAP, NFR, 4, 4)
    xyz = np.zeros((CAP, 34, 3), np.float32)
    xb = np.asarray(xyz_buf, np.float32).reshape(nslots, 4, G, NPG)
    for b, rows in enumerate(rowmap):
        for (a, atom, ip) in rows:
            xyz[:, atom, ip] = xb[b, a].reshape(CAP)
    return frames, xyz


def trace_program(slots):
    NB = len(slots)
    nc = bacc.Bacc("TRN2", target_bir_lowering=False, debug=False,
                   num_devices=NCORE)
    t0_d = nc.dram_tensor("t0", [128, 4 * NPG], F16, kind="ExternalInput")
    ct_d = nc.dram_tensor("ct", [64, NSTEP * NPG], F16, kind="ExternalInput")
    st_d = nc.dram_tensor("stt", [64, NSTEP * NPG], F16, kind="ExternalInput")
    lh_d = nc.dram_tensor("lhsT", [128, NSTEP * 128], F16,
                          kind="ExternalInput")
    lw_d = nc.dram_tensor("lhsT_sw", [128, NSTEP * 64], F16,
                          kind="ExternalInput")
    wx_d = nc.dram_tensor("wx", [128, NB * 128], F16, kind="ExternalInput")
    frames_d = nc.dram_tensor("frames", [NFR, 128, 4 * NPG], F16,
                              kind="ExternalOutput")
    xyz_d = nc.dram_tensor("xyz", [NB, 128, NPG], F16, kind="ExternalOutput")

    with tile.TileContext(nc) as tc, ExitStack() as ctx:
        const = ctx.enter_context(tc.tile_pool(name="const", bufs=1))
        st = ctx.enter_context(tc.tile_pool(name="state", bufs=NCHUNK))
        work = ctx.enter_context(tc.tile_pool(name="work", bufs=2))
        ps = ctx.enter_context(tc.tile_pool(name="psum", bufs=2, space="PSUM"))
        ps2 = ctx.enter_context(tc.tile_pool(name="psum2", bufs=2,
                                             space="PSUM"))

        lhsT_t = const.tile([128, NSTEP * 128], F16, tag="lhsT")
        nc.sync.dma_start(out=lhsT_t[:], in_=lh_d[:])
        lhsw_t = const.tile([128, NSTEP * 64], F16, tag="lhsw")
        nc.sync.dma_start(out=lhsw_t[:], in_=lw_d[:])
        wx_t = const.tile([128, NB * 128], F16, tag="wx")
        nc.sync.dma_start(out=wx_t[:], in_=wx_d[:])

        for q in range(NCHUNK):
            nsl = slice(q * NPC, (q + 1) * NPC)

            ct_t = st.tile([64, NSTEP * NPC], F16, tag="ct")
            stt_t = st.tile([64, NSTEP * NPC], F16, tag="st")
            nc.sync.dma_start(
                out=ct_t[:],
                in_=ct_d[:].rearrange("p (s n) -> p s n", s=NSTEP)[:, :, nsl])
            nc.sync.dma_start(
                out=stt_t[:],
                in_=st_d[:].rearrange("p (s n) -> p s n", s=NSTEP)[:, :, nsl])

            T = [st.tile([128, CHF], F16, tag=f"T{fr}", name=f"T{fr}_{q}")
                 for fr in range(NFR)]
            nc.sync.dma_start(
                out=T[0][:],
                in_=t0_d[:].rearrange("p (n i) -> p n i", i=4)[:, nsl, :])

            u_t = st.tile([64, CHF], F16, tag="u")
            v_t = st.tile([64, CHF], F16, tag="v")
            for s, (src, dst) in enumerate(STEPS):
                m1 = ps.tile([128, CHF], F32, tag="m1", name=f"m1_{q}_{s}")
                p2 = ps2.tile([64, CHF], F32, tag="p2", name=f"p2_{q}_{s}")
                lh = lhsT_t[:, s * 128:(s + 1) * 128]
                lw = lhsw_t[:, s * 64:(s + 1) * 64]
                for lo in range(0, CHF, 512):
                    hi = min(lo + 512, CHF)
                    nc.tensor.matmul(m1[:, lo:hi], lh, T[src][:, lo:hi],
                                     start=True, stop=True)
                    nc.tensor.matmul(p2[:, lo:hi], lw, T[src][:, lo:hi],
                                     start=True, stop=True)
                nc.scalar.copy(T[dst][:], m1[:])

                def cs3(t):
                    v = t[:, s * NPC:(s + 1) * NPC]
                    return v[:, :, None].broadcast_to([64, NPC, 4])

                def st3(x):
                    return x.rearrange("p (n i) -> p n i", i=4)

                nc.vector.tensor_mul(st3(v_t[:]), st3(T[dst][0:64]),
                                     cs3(ct_t))
                nc.vector.tensor_mul(st3(u_t[:]), st3(p2[:]), cs3(stt_t))
                nc.vector.tensor_add(T[dst][0:64], v_t[:], u_t[:])

            for fr in range(NFR):
                nc.gpsimd.dma_start(
                    out=frames_d[fr].rearrange("p (n i) -> p n i", i=4)
                        [:, nsl, :],
                    in_=T[fr][:])

            for g0 in range(0, NB, XG):
                xp = ps.tile([128, XG * NPC], F32, tag="m1",
                             name=f"xp_{q}_{g0}")
                for b in range(g0, g0 + XG):
                    fr, ip, _ = slots[b]
                    rhs = T[fr][:].rearrange("p (n i) -> p n i", i=4)[:, :, ip]
                    nc.tensor.matmul(
                        xp[:, (b - g0) * NPC:(b - g0 + 1) * NPC],
                        wx_t[:, b * 128:(b + 1) * 128], rhs,
                        start=True, stop=True)
                xs = work.tile([128, XG * NPC], F16, tag="xs")
                nc.scalar.copy(xs[:], xp[:])
                nc.gpsimd.dma_start(
                    out=xyz_d[g0:g0 + XG].rearrange("b p n -> p b n")
                        [:, :, nsl],
                    in_=xs[:].rearrange("p (b n) -> p b n", b=XG))
    nc.compile()
    return nc


def np_compute(idx, alphas, Rs, Ts, rts, xyzt, bi, seq):
    """Exact numpy path for residues the device capacity doesn't cover."""
    al = np.asarray(alphas[0], np.float32)[idx]
    n = np.sqrt((al ** 2).sum(-1)) + EPS
    c, s = al[..., 0] / n, al[..., 1] / n
    rot = np.zeros((len(idx), 10, 4, 4), np.float32)
    rot[:, :, 0, 0] = 1
    rot[:, :, 3, 3] = 1
    rot[:, :, 1, 1] = c
    rot[:, :, 1, 2] = -s
    rot[:, :, 2, 1] = s
    rot[:, :, 2, 2] = c
    sq = seq[idx]
    RT = np.asarray(rts, np.float32)[sq]
    RTF = np.zeros((len(idx), 9, 4, 4), np.float32)
    RTF[:, 0] = np.eye(4)
    RTF[:, 0, :3, :3] = np.asarray(Rs, np.float32)[idx]
    RTF[:, 0, :3, 3] = np.asarray(Ts, np.float32)[idx]
    for st_i, (src, dst) in enumerate(STEPS):
        k = st_i + 2
        RTF[:, dst] = RTF[:, src] @ RT[:, k] @ rot[:, k]
    bii = np.asarray(bi)[sq]
    per_atom = np.take_along_axis(RTF, bii[:, :, None, None], axis=1)
    basex = np.asarray(xyzt, np.float32)[sq]
    xyz = np.einsum('ltij,ltj->lti', per_atom, basex)
    return RTF, xyz[:, :, :3]


_CACHE = {}


def _get_program(slots):
    key = tuple(slots)
    if key not in _CACHE:
        _CACHE[key] = trace_program(slots)
    return _CACHE[key]


def prepare(inputs):
    alphas = np.asarray(inputs["alphas"])
    Rs = np.asarray(inputs["Rs"])
    Ts = np.asarray(inputs["Ts"])
    rts = np.asarray(inputs["RTs_in_base_frame"], np.float32)
    xyzt = np.asarray(inputs["xyzs_in_base_frame"], np.float32)
    bi = np.asarray(inputs["base_indices"]).astype(np.int64)
    seq = np.asarray(inputs["seq_index"]).astype(np.int64)[0]
    L = seq.shape[0]

    counts = np.bincount(seq, minlength=5)
    active = [t for t in range(5) if counts[t] > 0]
    share = counts[active] / L * NCORE
    ncores_t = np.maximum(1, np.floor(share).astype(int))
    while ncores_t.sum() < NCORE:
        ncores_t[np.argmax(share - ncores_t)] += 1
    while ncores_t.sum() > NCORE:
        cand = np.where(ncores_t > 1)[0]
        ncores_t[cand[np.argmax((ncores_t - share)[cand])]] -= 1

    slots, quads = make_schedule(bi, active)
    nc = _get_program(slots)

    tables = {t: build_type_tables(slots, quads[t], rts[t], xyzt[t])
              for t in active}

    in_maps = []
    core_meta = []
    host_idx = []
    for t, nct in zip(active, ncores_t):
        idx_t = np.where(seq == t)[0]
        per = min(CAP, -(-len(idx_t) // nct))
        for ci in range(nct):
            part = idx_t[ci * per:(ci + 1) * per]
            real = len(part)
            if real < CAP:
                pad = np.full(CAP - real, idx_t[0], dtype=part.dtype)
                part = np.concatenate([part, pad])
            m = pack_core_inputs(part, alphas, Rs, Ts)
            m["lhsT"], m["lhsT_sw"], m["wx"] = tables[t][0:3]
            in_maps.append(m)
            core_meta.append((t, part, real))
        if nct * per < len(idx_t):
            host_idx.append(idx_t[nct * per:])

    return nc, in_maps, core_meta, host_idx, tables, slots, L


def prepare_in_maps(inputs):
    return prepare(inputs)[1]


def run(inputs, trace=False):
    alphas = np.asarray(inputs["alphas"])
    Rs = np.asarray(inputs["Rs"])
    Ts = np.asarray(inputs["Ts"])
    rts = np.asarray(inputs["RTs_in_base_frame"], np.float32)
    xyzt = np.asarray(inputs["xyzs_in_base_frame"], np.float32)
    bi = np.asarray(inputs["base_indices"]).astype(np.int64)
    seq = np.asarray(inputs["seq_index"]).astype(np.int64)[0]
    nc, in_maps, core_meta, host_idx, tables, slots, L = prepare(inputs)
    res = run_bass_kernel_spmd(nc, in_maps, list(range(NCORE)), trace=trace)

    frames_full = np.zeros((L, NFR, 4, 4), np.float32)
    xyz_full = np.zeros((L, 34, 3), np.float32)
    for c, (t, part, real) in enumerate(core_meta):
        rowmap = tables[t][3]
        fr_c, xy_c = unpack_outputs(res.results[c]["frames"],
                                    res.results[c]["xyz"],
                                    len(slots), rowmap)
        frames_full[part[:real]] = fr_c[:real]
        xyz_full[part[:real]] = xy_c[:real]
    if host_idx:
        hidx = np.concatenate(host_idx)
        fr_h, xy_h = np_compute(hidx, alphas, Rs, Ts, rts, xyzt, bi, seq)
        frames_full[hidx] = fr_h
        xyz_full[hidx] = xy_h
    return (frames_full[None], xyz_full[None]), res


def kernel(**inputs):
    out, _ = run(inputs, trace=False)
    return out
